# revision 7
# baseline (speedup 1.0000x reference)
"""HeteroLightGCN on 8 Trainium2 NeuronCores (Bass/Tile).

Strategy (1D node partition, per sharding hint):
  - users/biz row-sharded across 8 cores (12500 / 2500 rows per core).
  - Each SpMM: dma_gather source rows (int16 idx; user table in 4 chunks
    of 25088 rows) from the AllGather'd full embedding table in HBM into a
    per-destination-window column grid (dst row on partition, its edges
    along free dim), DVE broadcast-multiply by edge weights, DVE reduce
    over edge slots, dma_scatter_add window sums into the new shard.
  - Degree-sorted windows keep grid fill ~96%.
  - 3 unrolled layers with AllGather of shards between layers; input
    projections on the PE; final L2 normalize on device.
"""
import sys

sys.path.insert(0, "/opt/trn_rl_repo")

import numpy as np
import concourse.bass as bass
import concourse.bacc as bacc
import concourse.tile as tile
import concourse.mybir as mybir
from concourse.bass_utils import run_bass_kernel_spmd

# ---------------- problem constants (hardcoded) ----------------
P = 128
F32 = mybir.dt.float32
I16 = mybir.dt.int16
SGRP = 16                                               # windows per scatter


def make_cfg(n_u=100000, n_b=20000, in_dim=384, d=128, layers=3, n_cores=8,
             n_uchunks=4):
    c = {}
    c["N_CORES"] = n_cores
    c["N_U"], c["N_B"] = n_u, n_b
    c["IN_DIM"], c["D"] = in_dim, d
    c["L"] = layers
    c["U_SH"], c["B_SH"] = n_u // n_cores, n_b // n_cores
    c["U_PAD"] = ((c["U_SH"] + P - 1) // P) * P
    c["B_PAD"] = ((c["B_SH"] + P - 1) // P) * P
    c["U_TAB"], c["B_TAB"] = c["U_PAD"] * n_cores, c["B_PAD"] * n_cores
    c["N_UCHUNKS"] = n_uchunks
    c["U_CHUNK"] = c["U_TAB"] // n_uchunks
    assert c["U_CHUNK"] % c["U_PAD"] == 0 and c["U_CHUNK"] < 32768
    assert c["B_TAB"] < 32768
    c["NW_U"], c["NW_B"] = c["U_PAD"] // P, c["B_PAD"] // P
    return c


CFG = make_cfg()

# ---------------- host-side prep ----------------


def _wrap_idx(flat):
    a = flat.reshape(-1, 16).T.astype(np.int16)
    return np.ascontiguousarray(np.tile(a, (8, 1)))


def _build_pass_core(src, dst_loc, val, n_rows_pad):
    """One (spmm, chunk) pass for one core. src already chunk-local."""
    cnt = np.bincount(dst_loc, minlength=n_rows_pad)
    perm = np.argsort(-cnt, kind="stable").astype(np.int32)
    pos = np.empty(n_rows_pad, np.int32)
    pos[perm] = np.arange(n_rows_pad, dtype=np.int32)
    epos = pos[dst_loc]
    order = np.argsort(epos, kind="stable")
    epos_s = epos[order]
    first = np.r_[0, np.flatnonzero(np.diff(epos_s)) + 1]
    reps = np.diff(np.r_[first, len(epos_s)])
    ccnt = (np.arange(len(epos_s)) - np.repeat(first, reps)).astype(np.int32)
    nw = n_rows_pad // P
    wcnt = cnt[perm].reshape(nw, P).max(axis=1).astype(np.int32)
    return dict(perm=perm, wcnt=wcnt, epos=epos_s, ccnt=ccnt,
                src=src[order], val=val[order], n_rows_pad=n_rows_pad)


def _finalize(p, wcols):
    col_off = np.r_[0, np.cumsum(wcols)][:-1].astype(np.int64)
    total_cols = int(wcols.sum())
    w = p["epos"] // P
    r = p["epos"] % P
    c_abs = col_off[w] + p["ccnt"]
    idx2d = np.zeros((P, total_cols), np.int32)
    val2d = np.zeros((P, total_cols), np.float32)
    idx2d[r, c_abs] = p["src"]
    val2d[r, c_abs] = p["val"]
    widx = _wrap_idx(idx2d.T.reshape(-1))
    sidx = _wrap_idx(p["perm"])
    return dict(widx=widx, val2d=val2d, sidx=sidx)


def host_prep(inp, cfg=CFG):
    """Returns (per_core_pass_arrays, meta). meta is shared across cores and
    defines the static program structure."""
    N_CORES = cfg["N_CORES"]; U_SH = cfg["U_SH"]; B_SH = cfg["B_SH"]
    U_PAD = cfg["U_PAD"]; B_PAD = cfg["B_PAD"]; U_TAB = cfg["U_TAB"]
    B_TAB = cfg["B_TAB"]; U_CHUNK = cfg["U_CHUNK"]; N_UCHUNKS = cfg["N_UCHUNKS"]
    ub_u = np.asarray(inp["ub_u"]).astype(np.int64)
    ub_b = np.asarray(inp["ub_b"]).astype(np.int64)
    uu_s = np.asarray(inp["uu_src"]).astype(np.int64)
    uu_d = np.asarray(inp["uu_dst"]).astype(np.int64)
    bb_s = np.asarray(inp["bb_src"]).astype(np.int64)
    bb_d = np.asarray(inp["bb_dst"]).astype(np.int64)

    def pad_u(u):
        return (u // U_SH) * U_PAD + (u % U_SH)

    def pad_b(b):
        return (b // B_SH) * B_PAD + (b % B_SH)

    spmms = [
        ("bu", "u", "b", pad_b(ub_b), ub_u, np.asarray(inp["val_bu"]), 1, B_TAB),
        ("uu", "u", "u", pad_u(uu_s), uu_d, np.asarray(inp["uu_val"]), N_UCHUNKS, U_CHUNK),
        ("ub", "b", "u", pad_u(ub_u), ub_b, np.asarray(inp["val_ub"]), N_UCHUNKS, U_CHUNK),
        ("bb", "b", "b", pad_b(bb_s), bb_d, np.asarray(inp["bb_val"]), 1, B_TAB),
    ]
    per_core = [[] for _ in range(N_CORES)]
    meta = []
    for name, side, table, src_pad, dst_nat, val, nch, chsz in spmms:
        sh, padn = (U_SH, U_PAD) if side == "u" else (B_SH, B_PAD)
        core_of = dst_nat // sh
        dst_loc = (dst_nat % sh).astype(np.int64)
        for k in range(nch):
            built = []
            for c in range(N_CORES):
                m = core_of == c
                if nch == 1:
                    s, d, v = src_pad[m], dst_loc[m], val[m]
                else:
                    mk = m & (src_pad >= k * chsz) & (src_pad < (k + 1) * chsz)
                    s, d, v = src_pad[mk] - k * chsz, dst_loc[mk], val[mk]
                built.append(_build_pass_core(s, d, v, padn))
            wcols = np.maximum.reduce([b["wcnt"] for b in built])
            wcols = np.maximum(wcols, 1)
            meta.append(dict(name=f"{name}{k}", side=side, table=table,
                             chunk=k, wcols=wcols))
            for c in range(N_CORES):
                per_core[c].append(_finalize(built[c], wcols))
    return per_core, meta


def _pack_calls(wcols, gmax):
    """Greedy-pack consecutive windows into gather calls of <= gmax cols.
    Returns list of (w_start, w_end, col_off)."""
    calls = []
    w = 0
    nw = len(wcols)
    off = 0
    while w < nw:
        cc = 0
        ws = w
        while w < nw and cc + wcols[w] <= gmax:
            cc += int(wcols[w])
            w += 1
        assert w > ws, f"window {w} cols {wcols[w]} > gmax {gmax}"
        calls.append((ws, w, off, cc))
        off += cc
    return calls


# ---------------- device program ----------------


def build_program(meta, cfg=CFG, stage=99):
    N_CORES = cfg["N_CORES"]; U_PAD = cfg["U_PAD"]; B_PAD = cfg["B_PAD"]
    U_TAB = cfg["U_TAB"]; B_TAB = cfg["B_TAB"]; U_CHUNK = cfg["U_CHUNK"]
    NW_U = cfg["NW_U"]; NW_B = cfg["NW_B"]; IN_DIM = cfg["IN_DIM"]
    D = cfg["D"]; L = cfg["L"]
    gmax = max(64, max(int(m["wcols"].max()) for m in meta))
    nc = bacc.Bacc("TRN2", target_bir_lowering=False, debug=False,
                   num_devices=N_CORES)

    featTu = nc.dram_tensor("featTu", [IN_DIM, U_PAD], F32, kind="ExternalInput").ap()
    featTb = nc.dram_tensor("featTb", [IN_DIM, B_PAD], F32, kind="ExternalInput").ap()
    w_u = nc.dram_tensor("w_u", [IN_DIM, D], F32, kind="ExternalInput").ap()
    w_b = nc.dram_tensor("w_b", [IN_DIM, D], F32, kind="ExternalInput").ap()
    pins = []
    for i, m in enumerate(meta):
        tc_total = int(m["wcols"].sum())
        nrp = U_PAD if m["side"] == "u" else B_PAD
        pins.append(dict(
            widx=nc.dram_tensor(f"widx{i}", [P, tc_total * 8], I16,
                                kind="ExternalInput").ap(),
            val=nc.dram_tensor(f"val{i}", [P, tc_total], F32,
                               kind="ExternalInput").ap(),
            sidx=nc.dram_tensor(f"sidx{i}", [P, nrp // 16], I16,
                                kind="ExternalInput").ap(),
        ))
    out_u = nc.dram_tensor("out_u", [U_PAD, D], F32, kind="ExternalOutput").ap()
    out_b = nc.dram_tensor("out_b", [B_PAD, D], F32, kind="ExternalOutput").ap()

    from contextlib import ExitStack
    with tile.TileContext(nc) as tc, ExitStack() as es:
        wp = es.enter_context(tc.tile_pool(name="wp", bufs=1))
        sump = es.enter_context(tc.tile_pool(name="sump", bufs=1))
        fp = es.enter_context(tc.tile_pool(name="fp", bufs=3))
        pp = es.enter_context(tc.tile_pool(name="pp", bufs=4, space="PSUM"))
        gp = es.enter_context(tc.tile_pool(name="gp", bufs=2))
        ip = es.enter_context(tc.tile_pool(name="ip", bufs=2))
        vp = es.enter_context(tc.tile_pool(name="vp", bufs=2))
        rp = es.enter_context(tc.tile_pool(name="rp", bufs=2))
        sp = es.enter_context(tc.tile_pool(name="sp", bufs=2))
        rb = es.enter_context(tc.tile_pool(name="rb", bufs=3))
        npool = es.enter_context(tc.tile_pool(name="npool", bufs=2))
        dp = es.enter_context(tc.tile_pool(name="dp", bufs=1, space="DRAM"))

        # --- constants / persistent ---
        wu_t = [wp.tile([P, D], F32, name=f"wu{k}", tag=f"wu{k}") for k in range(3)]
        wb_t = [wp.tile([P, D], F32, name=f"wb{k}", tag=f"wb{k}") for k in range(3)]
        for k in range(3):
            nc.sync.dma_start(out=wu_t[k][:], in_=w_u[k * P:(k + 1) * P, :])
            nc.sync.dma_start(out=wb_t[k][:], in_=w_b[k * P:(k + 1) * P, :])
        u_sum = sump.tile([P, NW_U, D], F32, name="u_sum", tag="u_sum")
        b_sum = sump.tile([P, NW_B, D], F32, name="b_sum", tag="b_sum")
        zt = sump.tile([P, 2048], F32, name="zt", tag="zt")
        nc.gpsimd.memset(zt[:], 0.0)

        u_cur = dp.tile([U_TAB, D], F32, tag="u_cur0", addr_space="Shared")
        b_cur = dp.tile([B_TAB, D], F32, tag="b_cur0", addr_space="Shared")
        u_sh0 = dp.tile([U_PAD, D], F32, name="u_sh0", tag="u_sh0")
        b_sh0 = dp.tile([B_PAD, D], F32, name="b_sh0", tag="b_sh0")

        # --- projections: sum_tile[:, w, :] = featT_tile.T @ W ---
        for side, featT, wt, nw, sumt, sh0 in (
            ("u", featTu, wu_t, NW_U, u_sum, u_sh0),
            ("b", featTb, wb_t, NW_B, b_sum, b_sh0),
        ):
            for w in range(nw):
                ps = pp.tile([P, D], F32, name="proj_ps", tag="proj_ps")
                for k in range(3):
                    ft = fp.tile([P, P], F32, name="ft", tag="ft")
                    nc.sync.dma_start(
                        out=ft[:], in_=featT[k * P:(k + 1) * P, w * P:(w + 1) * P])
                    nc.tensor.matmul(out=ps[:], lhsT=ft[:], rhs=wt[k][:],
                                     start=(k == 0), stop=(k == 2))
                nc.vector.tensor_copy(out=sumt[:, w, :], in_=ps[:])
                nc.sync.dma_start(out=sh0[w * P:(w + 1) * P, :], in_=sumt[:, w, :])

        rg = [list(range(N_CORES))]
        if stage >= 2:
            nc.gpsimd.collective_compute("AllGather", mybir.AluOpType.bypass,
                                         replica_groups=rg, ins=[u_sh0[:].opt()],
                                         outs=[u_cur[:].opt()])
            nc.gpsimd.collective_compute("AllGather", mybir.AluOpType.bypass,
                                         replica_groups=rg, ins=[b_sh0[:].opt()],
                                         outs=[b_cur[:].opt()])

        # --- layers ---
        n_layers = L if stage >= 5 else (1 if stage >= 3 else 0)
        for lyr in range(n_layers):
            u_new = dp.tile([U_PAD, D], F32, name=f"u_new{lyr}", tag=f"u_new{lyr}")
            b_new = dp.tile([B_PAD, D], F32, name=f"b_new{lyr}", tag=f"b_new{lyr}")
            # zero-init via zero tile (2048*128 elems per dma)
            for t, npad in ((u_new, U_PAD), (b_new, B_PAD)):
                r0 = 0
                while r0 < npad:
                    rr = min(2048, npad - r0)
                    nc.sync.dma_start(out=t[r0:r0 + rr, :],
                                      in_=zt[:, :rr * D // P])
                    r0 += rr

            for side, new_t, nw, sumt in (("u", u_new, NW_U, u_sum),
                                          ("b", b_new, NW_B, b_sum)):
                for i, m in enumerate(meta):
                    if m["side"] != side:
                        continue
                    if stage == 3 and not (side == "u" and m["chunk"] == 0
                                           and m["table"] == "b"):
                        continue
                    wcols = m["wcols"]
                    col_off = np.r_[0, np.cumsum(wcols)].astype(np.int64)
                    if m["table"] == "u":
                        tab = u_cur[m["chunk"] * U_CHUNK:
                                    (m["chunk"] + 1) * U_CHUNK, :]
                    else:
                        tab = b_cur[:, :]
                    calls = _pack_calls(wcols, gmax)
                    # scatter groups of SGRP windows
                    res_t = None
                    for (ws, we, coff, cc) in calls:
                        it = ip.tile([P, gmax * 8], I16, name="it", tag="it")
                        nc.sync.dma_start(
                            out=it[:, :cc * 8],
                            in_=pins[i]["widx"][:, coff * 8:(coff + cc) * 8])
                        vt = vp.tile([P, gmax], F32, name="vt", tag="vt")
                        nc.sync.dma_start(
                            out=vt[:, :cc],
                            in_=pins[i]["val"][:, coff:coff + cc])
                        gt = gp.tile([P, gmax, D], F32, name="gt", tag="gt")
                        nc.gpsimd.dma_gather(
                            out_ap=gt[:, :cc, :], in_ap=tab,
                            idxs_ap=it[:, :cc * 8], num_idxs=cc * P,
                            num_idxs_reg=cc * P, elem_size=D,
                            single_packet=False)
                        nc.vector.tensor_tensor(
                            out=gt[:, :cc, :], in0=gt[:, :cc, :],
                            in1=vt[:, :cc].to_broadcast([P, cc, D]),
                            op=mybir.AluOpType.mult)
                        for w in range(ws, we):
                            if w % SGRP == 0:
                                res_t = rp.tile([P, SGRP, D], F32, name="res", tag="res")
                            lo = int(col_off[w]) - coff
                            hi = int(col_off[w + 1]) - coff
                            nc.vector.tensor_reduce(
                                out=res_t[:, w % SGRP, :],
                                in_=gt[:, lo:hi, :].rearrange("p c d -> p d c"),
                                axis=mybir.AxisListType.X,
                                op=mybir.AluOpType.add)
                            if w % SGRP == SGRP - 1 or w == nw - 1:
                                g = w // SGRP
                                gwin = w % SGRP + 1
                                st = sp.tile([P, SGRP * 8], I16, name="st", tag="st")
                                nc.sync.dma_start(
                                    out=st[:, :gwin * 8],
                                    in_=pins[i]["sidx"][:, g * SGRP * 8:
                                                        g * SGRP * 8 + gwin * 8])
                                nc.gpsimd.dma_scatter_add(
                                    out_ap=new_t[:, :], in_ap=res_t[:, :gwin, :],
                                    idxs_ap=st[:, :gwin * 8],
                                    num_idxs=gwin * P, num_idxs_reg=gwin * P,
                                    elem_size=D, single_packet=False)
                # readback into running sum
                for w in range(nw):
                    rt = rb.tile([P, D], F32, name="rt", tag="rt")
                    nc.sync.dma_start(out=rt[:], in_=new_t[w * P:(w + 1) * P, :])
                    nc.vector.tensor_add(out=sumt[:, w, :], in0=sumt[:, w, :],
                                         in1=rt[:])
            if lyr < L - 1:
                u_cur = dp.tile([U_TAB, D], F32, name=f"u_cur{lyr + 1}", tag=f"u_cur{lyr + 1}",
                                addr_space="Shared")
                b_cur = dp.tile([B_TAB, D], F32, name=f"b_cur{lyr + 1}", tag=f"b_cur{lyr + 1}",
                                addr_space="Shared")
                nc.gpsimd.collective_compute(
                    "AllGather", mybir.AluOpType.bypass, replica_groups=rg,
                    ins=[u_new[:].opt()], outs=[u_cur[:].opt()])
                nc.gpsimd.collective_compute(
                    "AllGather", mybir.AluOpType.bypass, replica_groups=rg,
                    ins=[b_new[:].opt()], outs=[b_cur[:].opt()])

        # --- l2 normalize (x/max(||x||, eps); 1/(L+1) scale cancels, eps*4) ---
        if stage < 6:
            for sumt, nw, out_ext in ((u_sum, NW_U, out_u), (b_sum, NW_B, out_b)):
                for w in range(nw):
                    nc.sync.dma_start(out=out_ext[w * P:(w + 1) * P, :],
                                      in_=sumt[:, w, :])
        for sumt, nw, out_ext in (((u_sum, NW_U, out_u), (b_sum, NW_B, out_b))
                                  if stage >= 6 else ()):
            for w in range(nw):
                sq = npool.tile([P, D], F32, name="sq", tag="sq")
                n2 = npool.tile([P, 1], F32, name="n2", tag="n2")
                nc.scalar.activation(out=sq[:], in_=sumt[:, w, :],
                                     func=mybir.ActivationFunctionType.Square,
                                     accum_out=n2[:])
                nrm = npool.tile([P, 1], F32, name="nrm", tag="nrm")
                nc.scalar.activation(out=nrm[:], in_=n2[:],
                                     func=mybir.ActivationFunctionType.Sqrt)
                nc.vector.tensor_scalar_max(nrm[:], nrm[:], 4e-12)
                rcp = npool.tile([P, 1], F32, name="rcp", tag="rcp")
                nc.vector.reciprocal(rcp[:], nrm[:])
                res = npool.tile([P, D], F32, name="nres", tag="nres")
                nc.scalar.activation(out=res[:], in_=sumt[:, w, :],
                                     func=mybir.ActivationFunctionType.Copy,
                                     scale=rcp[:, :1])
                nc.sync.dma_start(out=out_ext[w * P:(w + 1) * P, :], in_=res[:])

    nc.compile()
    return nc


# ---------------- entry point ----------------


def _shard_featT(feat, sh, pad, n_cores=8):
    """[N, IN] -> per-core [IN, pad] transposed padded shards"""
    out = []
    for c in range(n_cores):
        blk = feat[c * sh:(c + 1) * sh]
        if pad > sh:
            blk = np.concatenate(
                [blk, np.zeros((pad - sh, feat.shape[1]), np.float32)])
        out.append(np.ascontiguousarray(blk.T))
    return out


def kernel(**inputs):
    cfg = CFG
    N_CORES = cfg["N_CORES"]; U_SH = cfg["U_SH"]; B_SH = cfg["B_SH"]
    U_PAD = cfg["U_PAD"]; B_PAD = cfg["B_PAD"]
    user_feat = np.asarray(inputs["user_feat"], np.float32)
    biz_feat = np.asarray(inputs["biz_feat"], np.float32)
    per_core, meta = host_prep(inputs, cfg)
    nc = build_program(meta, cfg)

    fu = _shard_featT(user_feat, U_SH, U_PAD, N_CORES)
    fb = _shard_featT(biz_feat, B_SH, B_PAD, N_CORES)
    in_maps = []
    for c in range(N_CORES):
        im = dict(featTu=fu[c], featTb=fb[c],
                  w_u=np.asarray(inputs["W_user"], np.float32),
                  w_b=np.asarray(inputs["W_biz"], np.float32))
        for i, pc in enumerate(per_core[c]):
            im[f"widx{i}"] = pc["widx"]
            im[f"val{i}"] = pc["val2d"]
            im[f"sidx{i}"] = pc["sidx"]
        in_maps.append(im)

    res = run_bass_kernel_spmd(nc, in_maps, list(range(N_CORES)))
    user_h = np.concatenate(
        [res.results[c]["out_u"][:U_SH] for c in range(N_CORES)])
    biz_h = np.concatenate(
        [res.results[c]["out_b"][:B_SH] for c in range(N_CORES)])
    return (user_h, biz_h)
